# revision 23
# baseline (speedup 1.0000x reference)
"""Trainium2 Bass kernel for multi-head attention with RoPE (bf16 v2).

Problem: B=2, S=2048, H=2048, 16 heads, head_dim=128.
  q/k/v = hidden @ W{q,k,v} + b{q,k,v}  (per-head reshape)
  q, k = rope(q), rope(k)   (interleaved rotate-half)
  out = softmax(q k^T / sqrt(hd)) v
  final = out @ Wo + bo

Sharding over 8 cores: core c = 4*b + g handles batch b and head group g
(4 heads = 512 of the 2048 hidden columns). Each core computes a partial
out-projection over its 512 columns; the host sums the 4 partials per batch
and adds the effective bias.

v2 changes vs baseline:
- All matmuls run in bf16 (under the 8-core DVFS clamp a 512-col matmul is
  259 ns in bf16 vs 272 ns in fp32r; bf16 also halves DMA bytes and SBUF).
- V bias eliminated: softmax rows sum to 1, so V's bias adds exactly
  bv @ Wo to every output row - folded into bo on the host.
- Softmax denominators: DVE pre-adds pairs of exp tiles, then 8 (not 16)
  ones-vector matmuls accumulate the row sums - halves the PE overhead of
  the rowsum pass; V-tile evictions moved to the Scalar engine to give the
  DVE room for the pair-adds.

Device-side layout (as baseline): host permutes Wq/Wk columns per head into
de-interleaved (even|odd) order so RoPE's rotate-half becomes a PE
half-swap permutation matmul plus two DVE multiplies; Q/K are produced
transposed ([head_dim, s]) directly by the projection matmuls so attention
scores need no transposes. Softmax skips max-subtraction (scores ~N(0,1)).
"""

import math
import os

import ml_dtypes
import numpy as np

os.environ.setdefault("JAX_COMPILATION_CACHE_DIR", "/tmp/jax_bass_cache")

import concourse.bass as bass  # noqa: E402
import concourse.mybir as mybir  # noqa: E402
import concourse.tile as tile  # noqa: E402
from concourse import bacc, bass_utils  # noqa: E402

try:
    import jax

    jax.config.update("jax_compilation_cache_dir", "/tmp/jax_bass_cache")
except Exception:
    pass

B, S, H = 2, 2048, 2048
NH, HD = 16, 128
NCORES = 8
HG = 4          # heads per core
DC = HG * HD    # 512 hidden columns per core
BASE = 10000.0

F32 = mybir.dt.float32
F32R = mybir.dt.float32r
BF16 = mybir.dt.bfloat16
NPBF = ml_dtypes.bfloat16
F8 = mybir.dt.float8e4
NPF8 = ml_dtypes.float8_e4m3fn

NS = S // 512      # 4 s-chunks of 512
NKT = S // 128     # 16 s tiles of 128
NHT = H // 128     # 16 contraction tiles of 128
SCALE = 1.0 / math.sqrt(HD)
# exp tiles are computed as exp(s*SCALE - ln 16): the 1/16 cancels in the
# softmax division but keeps fp8 pair sums far below the e4m3 max of 448
EXPB = -math.log(16.0)


def _r(ap):
    return ap.bitcast(F32R)


def _build_program():
    nc = bacc.Bacc("TRN2", target_bir_lowering=False, debug=False)

    # packed layouts (see kernel()): per-partition rows are DRAM-contiguous
    xTs = nc.dram_tensor("xTs", [128, NS * NHT * 512], BF16, kind="ExternalInput")
    wqp = nc.dram_tensor("wqp", [128, NHT * DC], BF16, kind="ExternalInput")
    wkp = nc.dram_tensor("wkp", [128, NHT * DC], BF16, kind="ExternalInput")
    wvp = nc.dram_tensor("wvp", [128, NHT * DC], BF16, kind="ExternalInput")
    # bqk columns 0..2HG-1: per-head q/k biases; columns 2HG..4HG-1: the same
    # biases with their 64-row halves swapped (for the rope-swapped eviction)
    bqk = nc.dram_tensor("bqk", [128, 4 * HG + 1], F32, kind="ExternalInput")
    cosT = nc.dram_tensor("cosT", [128, S], BF16, kind="ExternalInput")
    sinTs = nc.dram_tensor("sinTs", [128, S], BF16, kind="ExternalInput")
    od = nc.dram_tensor("od", [128, 2, 32], F8, kind="ExternalInput")
    wo = nc.dram_tensor("wo", [DC, H], BF16, kind="ExternalInput")
    out = nc.dram_tensor("out", [S, H], F32, kind="ExternalOutput")

    with tile.TileContext(nc) as tc:
        with tc.tile_pool(name="persist", bufs=1) as pp:
            qt = [pp.tile([128, S], BF16, tag=f"qt{h}", name=f"qt{h}") for h in range(HG)]
            kt = [pp.tile([128, S], BF16, tag=f"kt{h}", name=f"kt{h}") for h in range(HG)]
            vt = [pp.tile([128, DC], BF16, tag=f"vt{t}", name=f"vt{t}") for t in range(NKT)]
            bqk_sb = pp.tile([128, 4 * HG + 1], F32, tag="bqk", name="bqk_sb")
            od_sb = pp.tile([128, 2, 32], F8, tag="od", name="od_sb")
            wot = [pp.tile([128, H], BF16, tag=f"wo{dc}", name=f"wo{dc}")
                   for dc in range(HG)]

            # ---------------- phase 1: projections + rope ----------------
            with tc.tile_pool(name="ph1", bufs=1) as p1, \
                 tc.tile_pool(name="ph1ps", bufs=8, space="PSUM") as ps1:

                def load_w_slab(wdram, w, n, label, split=False):
                    # [128, 2048] slab = contraction blocks 4w..4w+3
                    t = p1.tile([128, 2048], BF16, tag="wslab", bufs=5,
                                name=f"w_{label}_{n}_{w}")
                    if split:
                        # fine granularity so the very first matmul starts as
                        # soon as its 128-col block lands
                        for q8 in range(8):
                            nc.sync.dma_start(
                                t[:, 256 * q8:256 * (q8 + 1)],
                                wdram[:, 2048 * w + 256 * q8:
                                      2048 * w + 256 * (q8 + 1)])
                    else:
                        nc.sync.dma_start(t[:], wdram[:, 2048 * w:2048 * (w + 1)])
                    return t

                def rope_evict(ps_acc, dst_slice, bcol, n, h, which,
                               cos_sb, sin_sb):
                    # two PSUM->SBUF evictions on the Scalar engine: the plain
                    # biased copy, and a half-swapped copy (partition-offset
                    # PSUM read with the half-swapped bias) - no PE matmul
                    q0 = p1.tile([128, 512], BF16, tag="q0", bufs=2,
                                 name=f"q0_{n}_{h}_{which}")
                    nc.scalar.activation(
                        q0[:], ps_acc[:],
                        mybir.ActivationFunctionType.Identity,
                        bias=bqk_sb[:, bcol:bcol + 1], scale=1.0)
                    qsw = p1.tile([128, 512], BF16, tag="qsw", bufs=2,
                                  name=f"qsw_{n}_{h}_{which}")
                    bsw = bcol + 2 * HG
                    nc.scalar.activation(
                        qsw[0:64, :], ps_acc[64:128, :],
                        mybir.ActivationFunctionType.Identity,
                        bias=bqk_sb[0:64, bsw:bsw + 1], scale=1.0)
                    nc.scalar.activation(
                        qsw[64:128, :], ps_acc[0:64, :],
                        mybir.ActivationFunctionType.Identity,
                        bias=bqk_sb[64:128, bsw:bsw + 1], scale=1.0)
                    # rope: dst = q0*cos + qsw*sin_signed
                    t1 = p1.tile([128, 512], BF16, tag="t1", bufs=2,
                                 name=f"t1_{n}_{h}_{which}")
                    nc.vector.tensor_tensor(t1[:], qsw[:], sin_sb[:],
                                            op=mybir.AluOpType.mult)
                    nc.vector.tensor_tensor(dst_slice, q0[:], cos_sb[:],
                                            op=mybir.AluOpType.mult)
                    nc.vector.tensor_tensor(dst_slice, dst_slice, t1[:],
                                            op=mybir.AluOpType.add)

                for n in range(NS):
                    if n == 1:
                        # out-projection weights ride the sync DMA queue here,
                        # well before phase 2 needs them
                        for dc in range(HG):
                            nc.sync.dma_start(wot[dc][:],
                                              wo[128 * dc:128 * (dc + 1), :])
                    # xT slabs for this chunk; rotating tags prefetch one
                    # slab of the next chunk while this one computes.
                    # xT slabs ride the gpsimd (SWDGE) ring so their slot
                    # waits never head-of-line-block the weight stream on sync
                    xslabs = []
                    for w in range(4):
                        xs = p1.tile([128, 2048], BF16, tag=f"xs{(4 * n + w) % 6}",
                                     bufs=1, name=f"xs_{n}_{w}")
                        base = 8192 * n + 2048 * w
                        if n == 0:
                            # quarter-granularity so the first matmuls start as
                            # soon as their block lands, not the whole slab
                            for q4 in range(4):
                                nc.gpsimd.dma_start(
                                    xs[:, 512 * q4:512 * (q4 + 1)],
                                    xTs[:, base + 512 * q4:base + 512 * (q4 + 1)])
                        else:
                            nc.gpsimd.dma_start(xs[:], xTs[:, base:base + 2048])
                        xslabs.append(xs)

                    def xt_block(ht):
                        s = xslabs[ht // 4]
                        return s[:, 512 * (ht % 4):512 * (ht % 4 + 1)]

                    def qk_pass(which, wdram, evict_accs, evict_base):
                        # one projection pass; optionally interleaves the
                        # previous pass's rope evictions at w boundaries so the
                        # PSUM ring and DVE work stay spread out
                        accs = [ps1.tile([128, 512], F32, tag="pp",
                                         name=f"acc{n}{which}{h}") for h in range(HG)]
                        for w in range(4):
                            wt = load_w_slab(wdram, w, n, f"qk{which}",
                                             split=(n == 0 and which == 0 and w == 0))
                            for hti in range(4):
                                ht = 4 * w + hti
                                for h in range(HG):
                                    nc.tensor.matmul(
                                        accs[h][:],
                                        wt[:, 512 * hti + 128 * h:
                                           512 * hti + 128 * (h + 1)],
                                        xt_block(ht),
                                        start=(ht == 0), stop=(ht == NHT - 1))
                            if evict_accs is not None:
                                dst = kt if evict_base else qt
                                bc = (HG + w) if evict_base else w
                                rope_evict(evict_accs[w],
                                           dst[w][:, 512 * n:512 * (n + 1)],
                                           bc, n, w, evict_base, cos_sb, sin_sb)
                        return accs

                    def load_tables(first_pass_done=None):
                        # constants and rope tables are first needed by the
                        # evictions; keep them off the critical startup path
                        # behind the first weight slabs
                        if n == 0:
                            nc.sync.dma_start(bqk_sb[:], bqk[:])
                            nc.sync.dma_start(od_sb[:], od[:])
                        cos_sb = p1.tile([128, 512], BF16, tag="cos", bufs=2,
                                         name=f"cos_{n}")
                        sin_sb = p1.tile([128, 512], BF16, tag="sin", bufs=2,
                                         name=f"sin_{n}")
                        ns = slice(512 * n, 512 * (n + 1))
                        nc.sync.dma_start(cos_sb[:], cosT[:, ns])
                        nc.sync.dma_start(sin_sb[:], sinTs[:, ns])
                        return cos_sb, sin_sb

                    def v_pass(kaccs_to_evict):
                        # V pass (natural [s, d] layout); K rope evictions (if
                        # any) interleave between weight-slab groups
                        vaccs = [ps1.tile([128, DC], F32, tag="pp",
                                          name=f"vacc{n}{t}") for t in range(4)]
                        for w in range(4):
                            wt = load_w_slab(wvp, w, n, "v")
                            for hti in range(4):
                                ht = 4 * w + hti
                                for t in range(4):
                                    nc.tensor.matmul(
                                        vaccs[t][:],
                                        xt_block(ht)[:, 128 * t:128 * (t + 1)],
                                        wt[:, 512 * hti:512 * (hti + 1)],
                                        start=(ht == 0), stop=(ht == NHT - 1))
                            if kaccs_to_evict is not None:
                                rope_evict(kaccs_to_evict[w],
                                           kt[w][:, 512 * n:512 * (n + 1)],
                                           HG + w, n, w, 1, cos_sb, sin_sb)
                        # V eviction on the Scalar engine (no bias needed: bv
                        # is folded into bo on the host since softmax rows sum
                        # to 1)
                        for t in range(4):
                            nc.scalar.activation(vt[4 * n + t][:], vaccs[t][:],
                                                 mybir.ActivationFunctionType.Copy)

                    if n < NS - 1:
                        # Q pass, K pass, Q evictions, V pass w/ K evictions
                        qaccs = qk_pass(0, wqp, None, 0)
                        cos_sb, sin_sb = load_tables()
                        kaccs = qk_pass(1, wkp, None, 1)
                        for h in range(HG):
                            rope_evict(qaccs[h], qt[h][:, 512 * n:512 * (n + 1)],
                                       h, n, h, 0, cos_sb, sin_sb)
                        v_pass(kaccs)
                    else:
                        # last chunk: K first so kt completes before the V
                        # pass - phase 2's first scores then follow the V
                        # matmuls with no boundary stall
                        kaccs = qk_pass(1, wkp, None, 1)
                        cos_sb, sin_sb = load_tables()
                        qaccs = qk_pass(0, wqp, kaccs, 1)
                        for h in range(HG):
                            rope_evict(qaccs[h], qt[h][:, 512 * n:512 * (n + 1)],
                                       h, n, h, 0, cos_sb, sin_sb)
                        v_pass(None)

            # ---------- phase 2+3: attention + output projection ----------
            with tc.tile_pool(name="ph2", bufs=1) as p2, \
                 tc.tile_pool(name="ph2ps", bufs=1, space="PSUM") as ps2:

                def norm_chain(po, pr, at_h, qc, h):
                    # at = po / rowsum: approx reciprocal (18 bits, plenty on
                    # top of bf16 matmuls), gpsimd partition-broadcast, DVE mult
                    r_sb = p2.tile([1, 512], F32, tag="r_sb", bufs=2,
                                   name=f"rsb{qc}{h}")
                    nc.vector.tensor_copy(r_sb[:], pr[0:1, :])
                    recip = p2.tile([1, 512], F32, tag="recip", bufs=2,
                                    name=f"recip{qc}{h}")
                    nc.vector.reciprocal_approx_fast(recip[:], r_sb[:])
                    rb = p2.tile([128, 512], F32, tag="rb", bufs=2,
                                 name=f"rbs{qc}{h}")
                    nc.gpsimd.partition_broadcast(rb[:], recip[:])
                    nc.vector.tensor_tensor(at_h[:], po[:], rb[:],
                                            op=mybir.AluOpType.mult)

                def emit_outproj_group(qc, ti, hc, ats, tag="pf"):
                    # one [128 q, 512 cols] block of the out-projection:
                    # 4 matmuls (contraction over this core's 512 d), DVE
                    # eviction, DMA out
                    rs_out = slice(512 * qc + 128 * ti,
                                   512 * qc + 128 * (ti + 1))
                    cs = slice(512 * hc, 512 * (hc + 1))
                    ps_f = ps2.tile([128, 512], F32, tag=tag,
                                    bufs=(1 if tag == "pf" else 2),
                                    name=f"pf{qc}{ti}{hc}")
                    for dc in range(HG):
                        nc.tensor.matmul(
                            ps_f[:], ats[dc][:, 128 * ti:128 * (ti + 1)],
                            wot[dc][:, cs],
                            start=(dc == 0), stop=(dc == HG - 1))
                    ost = p2.tile([128, 512], F32, tag="ost", bufs=3,
                                  name=f"ost{qc}{ti}{hc}")
                    nc.vector.tensor_copy(ost[:], ps_f[:])
                    nc.sync.dma_start(out[rs_out, cs], ost[:])

                # out-projection groups of chunk qc-1 are interleaved into the
                # exp-bound AV stream of chunk qc: the PE gets filler work that
                # never waits on the in-flight exp chain
                def make_emit(qc, h, exps):
                    # scores are emitted two k-tiles ahead of the exp/AV chain
                    # so the PE FIFO always holds work that does not depend on
                    # the in-flight exp
                    qs = slice(512 * qc, 512 * (qc + 1))

                    def emit_scores(kti):
                        ks = slice(128 * kti, 128 * (kti + 1))
                        ps_s = ps2.tile([128, 512], F32, tag="ps", bufs=3,
                                        name=f"pss{qc}{h}{kti}")
                        nc.tensor.matmul(ps_s[:], kt[h][:, ks],
                                         qt[h][:, qs], start=True, stop=True)
                        e = p2.tile([128, 512], BF16, tag="e", bufs=5,
                                    name=f"e{qc}{h}{kti}")
                        nc.scalar.activation(e[:], ps_s[:],
                                             mybir.ActivationFunctionType.Exp,
                                             scale=SCALE,
                                             bias=bqk_sb[:, 4 * HG:4 * HG + 1])
                        exps.append(e)

                    return emit_scores

                opq = []  # deferred out-proj groups: (qc, ti, hc, ats)
                blocks = [(qc, h) for qc in range(NS) for h in range(HG)]
                pending = None  # next block's prefetched (exps, emit)
                ats = []
                for bi, (qc, h) in enumerate(blocks):
                    if h == 0:
                        ats = []
                    hs = slice(128 * h, 128 * (h + 1))
                    ps_o = ps2.tile([128, 512], F32, tag="po", bufs=2,
                                    name=f"po{qc}{h}")
                    ps_r = ps2.tile([32, 512], F32, tag="pr", bufs=2,
                                    name=f"pr{qc}{h}")
                    pks = []
                    if pending is None:
                        exps = []
                        emit_scores = make_emit(qc, h, exps)
                        emit_scores(0)
                        emit_scores(1)
                    else:
                        exps, emit_scores = pending
                        pending = None
                    nxt = blocks[bi + 1] if bi + 1 < len(blocks) else None
                    # out-proj filler leads each kti-quad (the exp queue laps
                    # the PE at block starts); h==0 blocks trail it instead -
                    # their groups need the at tile normalized moments earlier
                    op_slot = 3 if h == 0 else 0
                    if True:
                        for kti in range(NKT):
                            if kti + 2 < NKT:
                                emit_scores(kti + 2)
                            elif nxt is not None:
                                # tail slots: prefetch the next block's first
                                # scores so its AV stream starts with hot exps
                                if kti == NKT - 2:
                                    nexps = []
                                    nemit = make_emit(nxt[0], nxt[1], nexps)
                                    nemit(0)
                                    pending = (nexps, nemit)
                                else:
                                    nemit(1)
                            if kti % 4 == op_slot and opq:
                                emit_outproj_group(*opq.pop(0))
                            e = exps[kti]
                            nc.tensor.matmul(ps_o[:], vt[kti][:, hs], e[:],
                                             start=(kti == 0), stop=(kti == NKT - 1))
                            if kti % 2 == 1:
                                # DVE pair-adds write fp8 into packed slots;
                                # every second pair closes a DoubleRow rowsum
                                # matmul (2 pairs = 256 k per pass, 0.5 cyc/row)
                                j = kti // 2
                                pk_i, slot = divmod(j, 2)
                                if slot == 0:
                                    pks.append(p2.tile(
                                        [128, 2, 512], F8, tag="pk8", bufs=3,
                                        name=f"pk{qc}{h}{pk_i}"))
                                nc.vector.tensor_tensor(
                                    pks[pk_i][:, slot, :], exps[kti - 1][:], e[:],
                                    op=mybir.AluOpType.add)
                                if slot == 1:
                                    nc.tensor.matmul(
                                        ps_r[:], od_sb[:], pks[pk_i][:],
                                        start=(pk_i == 0), stop=(pk_i == 3),
                                        perf_mode=mybir.MatmulPerfMode.DoubleRow)
                    at_h = p2.tile([128, 512], BF16, tag=f"at{h}", bufs=2,
                                   name=f"at{qc}{h}")
                    ats.append(at_h)
                    norm_chain(ps_o, ps_r, at_h, qc, h)
                    if h == HG - 1:
                        opq = [(qc, ti, hc, ats)
                               for ti in range(4) for hc in range(NS)]
                for i, g in enumerate(opq):
                    emit_outproj_group(*g, tag=("pf" if i % 2 == 0 else "po"))

    nc.finalize()
    return nc


_NC = None
LAST_RESULTS = None


def _rope_tables():
    j = np.arange(HD // 2, dtype=np.float64)
    inv_freq = BASE ** (-2.0 * j / HD)
    pos = np.arange(S, dtype=np.float64)
    ang = pos[None, :] * inv_freq[:, None]          # [64, S]
    cos = np.cos(ang)
    sin = np.sin(ang)
    cosT = np.concatenate([cos, cos], axis=0)       # [128, S]
    sinTs = np.concatenate([-sin, sin], axis=0)     # [128, S]
    return (np.ascontiguousarray(cosT.astype(NPBF)),
            np.ascontiguousarray(sinTs.astype(NPBF)))


def _pack_w(w):
    # [H, DC] -> [128, NHT*DC]: block j holds rows 128j..128j+127; each
    # SBUF partition's slab row is contiguous in DRAM
    return np.ascontiguousarray(
        w.reshape(NHT, 128, DC).transpose(1, 0, 2).reshape(128, NHT * DC)
        .astype(NPBF))


def kernel(hidden_state, Wq, bq, Wk, bk, Wv, bv, Wo, bo):
    global _NC, LAST_RESULTS
    hidden_state = np.asarray(hidden_state, dtype=np.float32)
    Wq, bq = np.asarray(Wq, np.float32), np.asarray(bq, np.float32)
    Wk, bk = np.asarray(Wk, np.float32), np.asarray(bk, np.float32)
    Wv, bv = np.asarray(Wv, np.float32), np.asarray(bv, np.float32)
    Wo, bo = np.asarray(Wo, np.float32), np.asarray(bo, np.float32)

    if _NC is None:
        _NC = _build_program()

    cosT, sinTs = _rope_tables()
    perm = np.concatenate([np.arange(0, HD, 2), np.arange(1, HD, 2)])
    od_v = np.ones((128, 2, 32), NPF8)

    in_maps = []
    for c in range(NCORES):
        b, g = divmod(c, HG)
        cols = np.arange(DC) + DC * g
        # per-head de-interleave permutation for Q/K columns
        pcols = np.concatenate([DC * g + HD * h + perm for h in range(HG)])
        bqk_m = np.concatenate([bq[pcols].reshape(HG, HD).T,
                                bk[pcols].reshape(HG, HD).T], axis=1)  # [128, 2*HG]
        # half-swapped copy for the rope-swapped eviction, then the exp
        # bias column (-ln 16, see EXPB)
        bqk_m = np.concatenate(
            [bqk_m, np.concatenate([bqk_m[64:128], bqk_m[0:64]], axis=0),
             np.full((128, 1), EXPB, np.float32)],
            axis=1)  # [128, 4*HG+1]
        # hidden^T packed: [s-chunk n][block j][s within chunk] contiguous
        # per partition: xTs[p, n, j, c] = hidden[b][512n+c, 128j+p]
        xT = hidden_state[b].T                       # [H, S]
        xTs = (xT.reshape(NHT, 128, NS, 512)
               .transpose(1, 2, 0, 3).reshape(128, NS * NHT * 512))
        in_maps.append({
            "xTs": np.ascontiguousarray(xTs.astype(NPBF)),
            "wqp": _pack_w(Wq[:, pcols]),
            "wkp": _pack_w(Wk[:, pcols]),
            "wvp": _pack_w(Wv[:, cols]),
            "bqk": np.ascontiguousarray(bqk_m),
            "cosT": cosT,
            "sinTs": sinTs,
            "od": od_v,
            "wo": np.ascontiguousarray(Wo[cols, :].astype(NPBF)),
        })

    trace = bool(os.environ.get("KERNEL_TRACE"))
    res = bass_utils.run_bass_kernel_spmd(_NC, in_maps, core_ids=list(range(NCORES)),
                                          trace=trace)
    LAST_RESULTS = res

    out = np.zeros((B, S, H), np.float32)
    for c in range(NCORES):
        b = c // HG
        out[b] += res.results[c]["out"]
    # bv's contribution: softmax rows sum to 1, so adding bv to V adds
    # exactly bv @ Wo to every output row
    out += (bo + bv.astype(np.float64) @ Wo.astype(np.float64)).astype(
        np.float32)[None, None, :]
    return out


# revision 24
# speedup vs baseline: 1.0630x; 1.0630x over previous
"""Trainium2 Bass kernel for multi-head attention with RoPE (bf16 v2).

Problem: B=2, S=2048, H=2048, 16 heads, head_dim=128.
  q/k/v = hidden @ W{q,k,v} + b{q,k,v}  (per-head reshape)
  q, k = rope(q), rope(k)   (interleaved rotate-half)
  out = softmax(q k^T / sqrt(hd)) v
  final = out @ Wo + bo

Sharding over 8 cores: core c = 4*b + g handles batch b and head group g
(4 heads = 512 of the 2048 hidden columns). Each core computes a partial
out-projection over its 512 columns; the host sums the 4 partials per batch
and adds the effective bias.

v2 changes vs baseline:
- All matmuls run in bf16 (under the 8-core DVFS clamp a 512-col matmul is
  259 ns in bf16 vs 272 ns in fp32r; bf16 also halves DMA bytes and SBUF).
- V bias eliminated: softmax rows sum to 1, so V's bias adds exactly
  bv @ Wo to every output row - folded into bo on the host.
- Softmax denominators: DVE pre-adds pairs of exp tiles, then 8 (not 16)
  ones-vector matmuls accumulate the row sums - halves the PE overhead of
  the rowsum pass; V-tile evictions moved to the Scalar engine to give the
  DVE room for the pair-adds.

Device-side layout (as baseline): host permutes Wq/Wk columns per head into
de-interleaved (even|odd) order so RoPE's rotate-half becomes a PE
half-swap permutation matmul plus two DVE multiplies; Q/K are produced
transposed ([head_dim, s]) directly by the projection matmuls so attention
scores need no transposes. Softmax skips max-subtraction (scores ~N(0,1)).
"""

import math
import os

import ml_dtypes
import numpy as np

os.environ.setdefault("JAX_COMPILATION_CACHE_DIR", "/tmp/jax_bass_cache")

import concourse.bass as bass  # noqa: E402
import concourse.mybir as mybir  # noqa: E402
import concourse.tile as tile  # noqa: E402
from concourse import bacc, bass_utils  # noqa: E402

try:
    import jax

    jax.config.update("jax_compilation_cache_dir", "/tmp/jax_bass_cache")
except Exception:
    pass

B, S, H = 2, 2048, 2048
NH, HD = 16, 128
NCORES = 8
HG = 4          # heads per core
DC = HG * HD    # 512 hidden columns per core
BASE = 10000.0

F32 = mybir.dt.float32
F32R = mybir.dt.float32r
BF16 = mybir.dt.bfloat16
NPBF = ml_dtypes.bfloat16
F8 = mybir.dt.float8e4
NPF8 = ml_dtypes.float8_e4m3fn

NS = S // 512      # 4 s-chunks of 512
NKT = S // 128     # 16 s tiles of 128
NHT = H // 128     # 16 contraction tiles of 128
SCALE = 1.0 / math.sqrt(HD)
# exp tiles are computed as exp(s*SCALE - ln 16): the 1/16 cancels in the
# softmax division but keeps fp8 pair sums far below the e4m3 max of 448
EXPB = -math.log(16.0)


def _r(ap):
    return ap.bitcast(F32R)


def _build_program():
    nc = bacc.Bacc("TRN2", target_bir_lowering=False, debug=False)

    # packed layouts (see kernel()): per-partition rows are DRAM-contiguous
    xTs = nc.dram_tensor("xTs", [128, NS * NHT * 512], BF16, kind="ExternalInput")
    wqp = nc.dram_tensor("wqp", [128, NHT * DC], BF16, kind="ExternalInput")
    wkp = nc.dram_tensor("wkp", [128, NHT * DC], BF16, kind="ExternalInput")
    wvp = nc.dram_tensor("wvp", [128, NHT * DC], BF16, kind="ExternalInput")
    # bqk columns 0..2HG-1: per-head q/k biases; columns 2HG..4HG-1: the same
    # biases with their 64-row halves swapped (for the rope-swapped eviction)
    bqk = nc.dram_tensor("bqk", [128, 4 * HG + 1], F32, kind="ExternalInput")
    cosT = nc.dram_tensor("cosT", [128, S], BF16, kind="ExternalInput")
    sinTs = nc.dram_tensor("sinTs", [128, S], BF16, kind="ExternalInput")
    od = nc.dram_tensor("od", [128, 2, 64], F8, kind="ExternalInput")
    wo = nc.dram_tensor("wo", [DC, H], BF16, kind="ExternalInput")
    out = nc.dram_tensor("out", [S, H], F32, kind="ExternalOutput")

    with tile.TileContext(nc) as tc:
        with tc.tile_pool(name="persist", bufs=1) as pp:
            qt = [pp.tile([128, S], BF16, tag=f"qt{h}", name=f"qt{h}") for h in range(HG)]
            kt = [pp.tile([128, S], BF16, tag=f"kt{h}", name=f"kt{h}") for h in range(HG)]
            vt = [pp.tile([128, DC], BF16, tag=f"vt{t}", name=f"vt{t}") for t in range(NKT)]
            bqk_sb = pp.tile([128, 4 * HG + 1], F32, tag="bqk", name="bqk_sb")
            od_sb = pp.tile([128, 2, 64], F8, tag="od", name="od_sb")
            wot = [pp.tile([128, H], BF16, tag=f"wo{dc}", name=f"wo{dc}")
                   for dc in range(HG)]

            # ---------------- phase 1: projections + rope ----------------
            with tc.tile_pool(name="ph1", bufs=1) as p1, \
                 tc.tile_pool(name="ph1ps", bufs=8, space="PSUM") as ps1:

                def load_w_slab(wdram, w, n, label, split=False):
                    # [128, 2048] slab = contraction blocks 4w..4w+3
                    t = p1.tile([128, 2048], BF16, tag="wslab", bufs=5,
                                name=f"w_{label}_{n}_{w}")
                    if split:
                        # fine granularity so the very first matmul starts as
                        # soon as its 128-col block lands
                        for q4 in range(4):
                            nc.sync.dma_start(
                                t[:, 512 * q4:512 * (q4 + 1)],
                                wdram[:, 2048 * w + 512 * q4:
                                      2048 * w + 512 * (q4 + 1)])
                    else:
                        nc.sync.dma_start(t[:], wdram[:, 2048 * w:2048 * (w + 1)])
                    return t

                def rope_evict(ps_acc, dst_slice, bcol, n, h, which,
                               cos_sb, sin_sb):
                    # two PSUM->SBUF evictions on the Scalar engine: the plain
                    # biased copy, and a half-swapped copy (partition-offset
                    # PSUM read with the half-swapped bias) - no PE matmul
                    q0 = p1.tile([128, 512], BF16, tag="q0", bufs=2,
                                 name=f"q0_{n}_{h}_{which}")
                    nc.scalar.activation(
                        q0[:], ps_acc[:],
                        mybir.ActivationFunctionType.Identity,
                        bias=bqk_sb[:, bcol:bcol + 1], scale=1.0)
                    qsw = p1.tile([128, 512], BF16, tag="qsw", bufs=2,
                                  name=f"qsw_{n}_{h}_{which}")
                    bsw = bcol + 2 * HG
                    nc.scalar.activation(
                        qsw[0:64, :], ps_acc[64:128, :],
                        mybir.ActivationFunctionType.Identity,
                        bias=bqk_sb[0:64, bsw:bsw + 1], scale=1.0)
                    nc.scalar.activation(
                        qsw[64:128, :], ps_acc[0:64, :],
                        mybir.ActivationFunctionType.Identity,
                        bias=bqk_sb[64:128, bsw:bsw + 1], scale=1.0)
                    # rope: dst = q0*cos + qsw*sin_signed
                    t1 = p1.tile([128, 512], BF16, tag="t1", bufs=2,
                                 name=f"t1_{n}_{h}_{which}")
                    nc.vector.tensor_tensor(t1[:], qsw[:], sin_sb[:],
                                            op=mybir.AluOpType.mult)
                    nc.vector.tensor_tensor(dst_slice, q0[:], cos_sb[:],
                                            op=mybir.AluOpType.mult)
                    nc.vector.tensor_tensor(dst_slice, dst_slice, t1[:],
                                            op=mybir.AluOpType.add)

                for n in range(NS):
                    if n == 1:
                        # out-projection weights ride the sync DMA queue here,
                        # well before phase 2 needs them
                        for dc in range(HG):
                            nc.sync.dma_start(wot[dc][:],
                                              wo[128 * dc:128 * (dc + 1), :])
                    # xT slabs for this chunk; rotating tags prefetch one
                    # slab of the next chunk while this one computes.
                    # xT slabs ride the gpsimd (SWDGE) ring so their slot
                    # waits never head-of-line-block the weight stream on sync
                    xslabs = []
                    for w in range(4):
                        xs = p1.tile([128, 2048], BF16, tag=f"xs{(4 * n + w) % 6}",
                                     bufs=1, name=f"xs_{n}_{w}")
                        base = 8192 * n + 2048 * w
                        if n == 0:
                            # quarter-granularity so the first matmuls start as
                            # soon as their block lands, not the whole slab
                            for q4 in range(4):
                                nc.gpsimd.dma_start(
                                    xs[:, 512 * q4:512 * (q4 + 1)],
                                    xTs[:, base + 512 * q4:base + 512 * (q4 + 1)])
                        else:
                            nc.gpsimd.dma_start(xs[:], xTs[:, base:base + 2048])
                        xslabs.append(xs)

                    def xt_block(ht):
                        s = xslabs[ht // 4]
                        return s[:, 512 * (ht % 4):512 * (ht % 4 + 1)]

                    def qk_pass(which, wdram, evict_accs, evict_base):
                        # one projection pass; optionally interleaves the
                        # previous pass's rope evictions at w boundaries so the
                        # PSUM ring and DVE work stay spread out
                        accs = [ps1.tile([128, 512], F32, tag="pp",
                                         name=f"acc{n}{which}{h}") for h in range(HG)]
                        for w in range(4):
                            wt = load_w_slab(wdram, w, n, f"qk{which}",
                                             split=(n == 0 and which == 0 and w == 0))
                            for hti in range(4):
                                ht = 4 * w + hti
                                for h in range(HG):
                                    nc.tensor.matmul(
                                        accs[h][:],
                                        wt[:, 512 * hti + 128 * h:
                                           512 * hti + 128 * (h + 1)],
                                        xt_block(ht),
                                        start=(ht == 0), stop=(ht == NHT - 1))
                            if evict_accs is not None:
                                dst = kt if evict_base else qt
                                bc = (HG + w) if evict_base else w
                                rope_evict(evict_accs[w],
                                           dst[w][:, 512 * n:512 * (n + 1)],
                                           bc, n, w, evict_base, cos_sb, sin_sb)
                        return accs

                    def load_tables(first_pass_done=None):
                        # constants and rope tables are first needed by the
                        # evictions; keep them off the critical startup path
                        # behind the first weight slabs
                        if n == 0:
                            nc.sync.dma_start(bqk_sb[:], bqk[:])
                            nc.sync.dma_start(od_sb[:], od[:])
                        cos_sb = p1.tile([128, 512], BF16, tag="cos", bufs=2,
                                         name=f"cos_{n}")
                        sin_sb = p1.tile([128, 512], BF16, tag="sin", bufs=2,
                                         name=f"sin_{n}")
                        ns = slice(512 * n, 512 * (n + 1))
                        nc.sync.dma_start(cos_sb[:], cosT[:, ns])
                        nc.sync.dma_start(sin_sb[:], sinTs[:, ns])
                        return cos_sb, sin_sb

                    def v_pass(kaccs_to_evict):
                        # V pass (natural [s, d] layout); K rope evictions (if
                        # any) interleave between weight-slab groups
                        vaccs = [ps1.tile([128, DC], F32, tag="pp",
                                          name=f"vacc{n}{t}") for t in range(4)]
                        for w in range(4):
                            wt = load_w_slab(wvp, w, n, "v")
                            for hti in range(4):
                                ht = 4 * w + hti
                                for t in range(4):
                                    nc.tensor.matmul(
                                        vaccs[t][:],
                                        xt_block(ht)[:, 128 * t:128 * (t + 1)],
                                        wt[:, 512 * hti:512 * (hti + 1)],
                                        start=(ht == 0), stop=(ht == NHT - 1))
                            if kaccs_to_evict is not None:
                                rope_evict(kaccs_to_evict[w],
                                           kt[w][:, 512 * n:512 * (n + 1)],
                                           HG + w, n, w, 1, cos_sb, sin_sb)
                        # V eviction on the Scalar engine (no bias needed: bv
                        # is folded into bo on the host since softmax rows sum
                        # to 1)
                        for t in range(4):
                            nc.scalar.activation(vt[4 * n + t][:], vaccs[t][:],
                                                 mybir.ActivationFunctionType.Copy)

                    if n < NS - 1:
                        # Q pass, K pass, Q evictions, V pass w/ K evictions
                        qaccs = qk_pass(0, wqp, None, 0)
                        cos_sb, sin_sb = load_tables()
                        kaccs = qk_pass(1, wkp, None, 1)
                        for h in range(HG):
                            rope_evict(qaccs[h], qt[h][:, 512 * n:512 * (n + 1)],
                                       h, n, h, 0, cos_sb, sin_sb)
                        v_pass(kaccs)
                    else:
                        # last chunk: K first so kt completes before the V
                        # pass - phase 2's first scores then follow the V
                        # matmuls with no boundary stall
                        kaccs = qk_pass(1, wkp, None, 1)
                        cos_sb, sin_sb = load_tables()
                        qaccs = qk_pass(0, wqp, kaccs, 1)
                        for h in range(HG):
                            rope_evict(qaccs[h], qt[h][:, 512 * n:512 * (n + 1)],
                                       h, n, h, 0, cos_sb, sin_sb)
                        v_pass(None)

            # ---------- phase 2+3: attention + output projection ----------
            with tc.tile_pool(name="ph2", bufs=1) as p2, \
                 tc.tile_pool(name="ph2ps", bufs=1, space="PSUM") as ps2:

                def norm_chain(po, pr, at_h, qc, h):
                    # at = po / rowsum. The DoubleRow rowsum already emitted 64
                    # replicated rows, so the reciprocal reads PSUM directly
                    # and two partition-offset half-multiplies normalize po -
                    # no cross-partition broadcast anywhere
                    recip = p2.tile([64, 512], F32, tag="recip", bufs=2,
                                    name=f"recip{qc}{h}")
                    nc.vector.reciprocal_approx_fast(recip[:], pr[0:64, :])
                    nc.vector.tensor_tensor(at_h[0:64, :], po[0:64, :],
                                            recip[:], op=mybir.AluOpType.mult)
                    nc.vector.tensor_tensor(at_h[64:128, :], po[64:128, :],
                                            recip[:], op=mybir.AluOpType.mult)

                def emit_outproj_group(qc, ti, hc, ats, tag="pf"):
                    # evictions alternate DVE/ACT to keep either queue shallow
                    # one [128 q, 512 cols] block of the out-projection:
                    # 4 matmuls (contraction over this core's 512 d), DVE
                    # eviction, DMA out
                    rs_out = slice(512 * qc + 128 * ti,
                                   512 * qc + 128 * (ti + 1))
                    cs = slice(512 * hc, 512 * (hc + 1))
                    ps_f = ps2.tile([128, 512], F32, tag=tag,
                                    bufs=(1 if tag == "pf" else 2),
                                    name=f"pf{qc}{ti}{hc}")
                    for dc in range(HG):
                        nc.tensor.matmul(
                            ps_f[:], ats[dc][:, 128 * ti:128 * (ti + 1)],
                            wot[dc][:, cs],
                            start=(dc == 0), stop=(dc == HG - 1))
                    ost = p2.tile([128, 512], F32, tag="ost", bufs=3,
                                  name=f"ost{qc}{ti}{hc}")
                    if hc % 2 == 0:
                        nc.vector.tensor_copy(ost[:], ps_f[:])
                    else:
                        nc.scalar.activation(ost[:], ps_f[:],
                                             mybir.ActivationFunctionType.Copy)
                    nc.sync.dma_start(out[rs_out, cs], ost[:])

                # out-projection groups of chunk qc-1 are interleaved into the
                # exp-bound AV stream of chunk qc: the PE gets filler work that
                # never waits on the in-flight exp chain
                def make_emit(qc, h, exps):
                    # scores are emitted two k-tiles ahead of the exp/AV chain
                    # so the PE FIFO always holds work that does not depend on
                    # the in-flight exp
                    qs = slice(512 * qc, 512 * (qc + 1))

                    def emit_scores(kti):
                        ks = slice(128 * kti, 128 * (kti + 1))
                        ps_s = ps2.tile([128, 512], F32, tag="ps", bufs=3,
                                        name=f"pss{qc}{h}{kti}")
                        nc.tensor.matmul(ps_s[:], kt[h][:, ks],
                                         qt[h][:, qs], start=True, stop=True)
                        e = p2.tile([128, 512], BF16, tag="e", bufs=5,
                                    name=f"e{qc}{h}{kti}")
                        nc.scalar.activation(e[:], ps_s[:],
                                             mybir.ActivationFunctionType.Exp,
                                             scale=SCALE,
                                             bias=bqk_sb[:, 4 * HG:4 * HG + 1])
                        exps.append(e)

                    return emit_scores

                opq = []  # deferred out-proj groups: (qc, ti, hc, ats)
                blocks = [(qc, h) for qc in range(NS) for h in range(HG)]
                pending = None  # next block's prefetched (exps, emit)
                ats = []
                for bi, (qc, h) in enumerate(blocks):
                    if h == 0:
                        ats = []
                    hs = slice(128 * h, 128 * (h + 1))
                    ps_o = ps2.tile([128, 512], F32, tag="po", bufs=2,
                                    name=f"po{qc}{h}")
                    ps_r = ps2.tile([64, 512], F32, tag="pr", bufs=2,
                                    name=f"pr{qc}{h}")
                    pks = []
                    if pending is None:
                        exps = []
                        emit_scores = make_emit(qc, h, exps)
                        emit_scores(0)
                        emit_scores(1)
                    else:
                        exps, emit_scores = pending
                        pending = None
                    nxt = blocks[bi + 1] if bi + 1 < len(blocks) else None
                    # out-proj filler leads each kti-quad (the exp queue laps
                    # the PE at block starts); h==0 blocks trail it instead -
                    # their groups need the at tile normalized moments earlier
                    op_slot = 3 if h == 0 else 0
                    if True:
                        for kti in range(NKT):
                            if kti + 2 < NKT:
                                emit_scores(kti + 2)
                            elif nxt is not None:
                                # tail slots: prefetch the next block's first
                                # scores so its AV stream starts with hot exps
                                if kti == NKT - 2:
                                    nexps = []
                                    nemit = make_emit(nxt[0], nxt[1], nexps)
                                    nemit(0)
                                    pending = (nexps, nemit)
                                else:
                                    nemit(1)
                            if kti % 4 == op_slot and opq:
                                emit_outproj_group(*opq.pop(0))
                            e = exps[kti]
                            nc.tensor.matmul(ps_o[:], vt[kti][:, hs], e[:],
                                             start=(kti == 0), stop=(kti == NKT - 1))
                            if kti % 2 == 1:
                                # DVE pair-adds write fp8 into packed slots;
                                # every second pair closes a DoubleRow rowsum
                                # matmul (2 pairs = 256 k per pass, 0.5 cyc/row)
                                j = kti // 2
                                pk_i, slot = divmod(j, 2)
                                if slot == 0:
                                    pks.append(p2.tile(
                                        [128, 2, 512], F8, tag="pk8", bufs=3,
                                        name=f"pk{qc}{h}{pk_i}"))
                                nc.vector.tensor_tensor(
                                    pks[pk_i][:, slot, :], exps[kti - 1][:], e[:],
                                    op=mybir.AluOpType.add)
                                if slot == 1:
                                    nc.tensor.matmul(
                                        ps_r[:], od_sb[:], pks[pk_i][:],
                                        start=(pk_i == 0), stop=(pk_i == 3),
                                        perf_mode=mybir.MatmulPerfMode.DoubleRow)
                    at_h = p2.tile([128, 512], BF16, tag=f"at{h}", bufs=2,
                                   name=f"at{qc}{h}")
                    ats.append(at_h)
                    norm_chain(ps_o, ps_r, at_h, qc, h)
                    if h == HG - 1:
                        opq = [(qc, ti, hc, ats)
                               for ti in range(4) for hc in range(NS)]
                for i, g in enumerate(opq):
                    emit_outproj_group(*g, tag=("pf" if i % 2 == 0 else "po"))

    nc.finalize()
    return nc


_NC = None
LAST_RESULTS = None


def _rope_tables():
    j = np.arange(HD // 2, dtype=np.float64)
    inv_freq = BASE ** (-2.0 * j / HD)
    pos = np.arange(S, dtype=np.float64)
    ang = pos[None, :] * inv_freq[:, None]          # [64, S]
    cos = np.cos(ang)
    sin = np.sin(ang)
    cosT = np.concatenate([cos, cos], axis=0)       # [128, S]
    sinTs = np.concatenate([-sin, sin], axis=0)     # [128, S]
    return (np.ascontiguousarray(cosT.astype(NPBF)),
            np.ascontiguousarray(sinTs.astype(NPBF)))


def _pack_w(w):
    # [H, DC] -> [128, NHT*DC]: block j holds rows 128j..128j+127; each
    # SBUF partition's slab row is contiguous in DRAM
    return np.ascontiguousarray(
        w.reshape(NHT, 128, DC).transpose(1, 0, 2).reshape(128, NHT * DC)
        .astype(NPBF))


def kernel(hidden_state, Wq, bq, Wk, bk, Wv, bv, Wo, bo):
    global _NC, LAST_RESULTS
    hidden_state = np.asarray(hidden_state, dtype=np.float32)
    Wq, bq = np.asarray(Wq, np.float32), np.asarray(bq, np.float32)
    Wk, bk = np.asarray(Wk, np.float32), np.asarray(bk, np.float32)
    Wv, bv = np.asarray(Wv, np.float32), np.asarray(bv, np.float32)
    Wo, bo = np.asarray(Wo, np.float32), np.asarray(bo, np.float32)

    if _NC is None:
        _NC = _build_program()

    cosT, sinTs = _rope_tables()
    perm = np.concatenate([np.arange(0, HD, 2), np.arange(1, HD, 2)])
    od_v = np.ones((128, 2, 64), NPF8)

    in_maps = []
    for c in range(NCORES):
        b, g = divmod(c, HG)
        cols = np.arange(DC) + DC * g
        # per-head de-interleave permutation for Q/K columns
        pcols = np.concatenate([DC * g + HD * h + perm for h in range(HG)])
        bqk_m = np.concatenate([bq[pcols].reshape(HG, HD).T,
                                bk[pcols].reshape(HG, HD).T], axis=1)  # [128, 2*HG]
        # half-swapped copy for the rope-swapped eviction, then the exp
        # bias column (-ln 16, see EXPB)
        bqk_m = np.concatenate(
            [bqk_m, np.concatenate([bqk_m[64:128], bqk_m[0:64]], axis=0),
             np.full((128, 1), EXPB, np.float32)],
            axis=1)  # [128, 4*HG+1]
        # hidden^T packed: [s-chunk n][block j][s within chunk] contiguous
        # per partition: xTs[p, n, j, c] = hidden[b][512n+c, 128j+p]
        xT = hidden_state[b].T                       # [H, S]
        xTs = (xT.reshape(NHT, 128, NS, 512)
               .transpose(1, 2, 0, 3).reshape(128, NS * NHT * 512))
        in_maps.append({
            "xTs": np.ascontiguousarray(xTs.astype(NPBF)),
            "wqp": _pack_w(Wq[:, pcols]),
            "wkp": _pack_w(Wk[:, pcols]),
            "wvp": _pack_w(Wv[:, cols]),
            "bqk": np.ascontiguousarray(bqk_m),
            "cosT": cosT,
            "sinTs": sinTs,
            "od": od_v,
            "wo": np.ascontiguousarray(Wo[cols, :].astype(NPBF)),
        })

    trace = bool(os.environ.get("KERNEL_TRACE"))
    res = bass_utils.run_bass_kernel_spmd(_NC, in_maps, core_ids=list(range(NCORES)),
                                          trace=trace)
    LAST_RESULTS = res

    out = np.zeros((B, S, H), np.float32)
    for c in range(NCORES):
        b = c // HG
        out[b] += res.results[c]["out"]
    # bv's contribution: softmax rows sum to 1, so adding bv to V adds
    # exactly bv @ Wo to every output row
    out += (bo + bv.astype(np.float64) @ Wo.astype(np.float64)).astype(
        np.float32)[None, None, :]
    return out


# revision 25
# speedup vs baseline: 1.0774x; 1.0135x over previous
"""Trainium2 Bass kernel for multi-head attention with RoPE (bf16 v2).

Problem: B=2, S=2048, H=2048, 16 heads, head_dim=128.
  q/k/v = hidden @ W{q,k,v} + b{q,k,v}  (per-head reshape)
  q, k = rope(q), rope(k)   (interleaved rotate-half)
  out = softmax(q k^T / sqrt(hd)) v
  final = out @ Wo + bo

Sharding over 8 cores: core c = 4*b + g handles batch b and head group g
(4 heads = 512 of the 2048 hidden columns). Each core computes a partial
out-projection over its 512 columns; the host sums the 4 partials per batch
and adds the effective bias.

v2 changes vs baseline:
- All matmuls run in bf16 (under the 8-core DVFS clamp a 512-col matmul is
  259 ns in bf16 vs 272 ns in fp32r; bf16 also halves DMA bytes and SBUF).
- V bias eliminated: softmax rows sum to 1, so V's bias adds exactly
  bv @ Wo to every output row - folded into bo on the host.
- Softmax denominators: DVE pre-adds pairs of exp tiles, then 8 (not 16)
  ones-vector matmuls accumulate the row sums - halves the PE overhead of
  the rowsum pass; V-tile evictions moved to the Scalar engine to give the
  DVE room for the pair-adds.

Device-side layout (as baseline): host permutes Wq/Wk columns per head into
de-interleaved (even|odd) order so RoPE's rotate-half becomes a PE
half-swap permutation matmul plus two DVE multiplies; Q/K are produced
transposed ([head_dim, s]) directly by the projection matmuls so attention
scores need no transposes. Softmax skips max-subtraction (scores ~N(0,1)).
"""

import math
import os

import ml_dtypes
import numpy as np

os.environ.setdefault("JAX_COMPILATION_CACHE_DIR", "/tmp/jax_bass_cache")

import concourse.bass as bass  # noqa: E402
import concourse.mybir as mybir  # noqa: E402
import concourse.tile as tile  # noqa: E402
from concourse import bacc, bass_utils  # noqa: E402

try:
    import jax

    jax.config.update("jax_compilation_cache_dir", "/tmp/jax_bass_cache")
except Exception:
    pass

B, S, H = 2, 2048, 2048
NH, HD = 16, 128
NCORES = 8
HG = 4          # heads per core
DC = HG * HD    # 512 hidden columns per core
BASE = 10000.0

F32 = mybir.dt.float32
F32R = mybir.dt.float32r
BF16 = mybir.dt.bfloat16
NPBF = ml_dtypes.bfloat16
F8 = mybir.dt.float8e4
NPF8 = ml_dtypes.float8_e4m3fn

NS = S // 512      # 4 s-chunks of 512
NKT = S // 128     # 16 s tiles of 128
NHT = H // 128     # 16 contraction tiles of 128
SCALE = 1.0 / math.sqrt(HD)
# exp tiles are computed as exp(s*SCALE - ln 16): the 1/16 cancels in the
# softmax division but keeps fp8 pair sums far below the e4m3 max of 448
EXPB = -math.log(16.0)


def _r(ap):
    return ap.bitcast(F32R)


def _build_program():
    nc = bacc.Bacc("TRN2", target_bir_lowering=False, debug=False)

    # packed layouts (see kernel()): per-partition rows are DRAM-contiguous
    xTs = nc.dram_tensor("xTs", [128, NS * NHT * 512], BF16, kind="ExternalInput")
    wqp = nc.dram_tensor("wqp", [128, NHT * DC], BF16, kind="ExternalInput")
    wkp = nc.dram_tensor("wkp", [128, NHT * DC], BF16, kind="ExternalInput")
    wvp = nc.dram_tensor("wvp", [128, NHT * DC], BF16, kind="ExternalInput")
    # bqk columns 0..2HG-1: per-head q/k biases; columns 2HG..4HG-1: the same
    # biases with their 64-row halves swapped (for the rope-swapped eviction)
    bqk = nc.dram_tensor("bqk", [128, 4 * HG + 1], F32, kind="ExternalInput")
    cosT = nc.dram_tensor("cosT", [128, S], BF16, kind="ExternalInput")
    sinTs = nc.dram_tensor("sinTs", [128, S], BF16, kind="ExternalInput")
    od = nc.dram_tensor("od", [128, 2, 64], F8, kind="ExternalInput")
    wo = nc.dram_tensor("wo", [DC, H], BF16, kind="ExternalInput")
    out = nc.dram_tensor("out", [S, H], F32, kind="ExternalOutput")

    with tile.TileContext(nc) as tc:
        with tc.tile_pool(name="persist", bufs=1) as pp:
            qt = [pp.tile([128, S], BF16, tag=f"qt{h}", name=f"qt{h}") for h in range(HG)]
            kt = [pp.tile([128, S], BF16, tag=f"kt{h}", name=f"kt{h}") for h in range(HG)]
            vt = [pp.tile([128, DC], BF16, tag=f"vt{t}", name=f"vt{t}") for t in range(NKT)]
            bqk_sb = pp.tile([128, 4 * HG + 1], F32, tag="bqk", name="bqk_sb")
            od_sb = pp.tile([128, 2, 64], F8, tag="od", name="od_sb")
            wot = [pp.tile([128, H], BF16, tag=f"wo{dc}", name=f"wo{dc}")
                   for dc in range(HG)]

            # ---------------- phase 1: projections + rope ----------------
            with tc.tile_pool(name="ph1", bufs=1) as p1, \
                 tc.tile_pool(name="ph1ps", bufs=8, space="PSUM") as ps1:

                def load_w_slab(wdram, w, n, label, split=False):
                    # [128, 2048] slab = contraction blocks 4w..4w+3
                    t = p1.tile([128, 2048], BF16, tag="wslab", bufs=5,
                                name=f"w_{label}_{n}_{w}")
                    if split:
                        # fine granularity so the very first matmul starts as
                        # soon as its 128-col block lands
                        for q4 in range(4):
                            nc.sync.dma_start(
                                t[:, 512 * q4:512 * (q4 + 1)],
                                wdram[:, 2048 * w + 512 * q4:
                                      2048 * w + 512 * (q4 + 1)])
                    else:
                        nc.sync.dma_start(t[:], wdram[:, 2048 * w:2048 * (w + 1)])
                    return t

                def rope_evict(ps_acc, dst_slice, bcol, n, h, which,
                               cos_sb, sin_sb):
                    # two PSUM->SBUF evictions on the Scalar engine: the plain
                    # biased copy, and a half-swapped copy (partition-offset
                    # PSUM read with the half-swapped bias) - no PE matmul
                    q0 = p1.tile([128, 512], BF16, tag="q0", bufs=2,
                                 name=f"q0_{n}_{h}_{which}")
                    nc.scalar.activation(
                        q0[:], ps_acc[:],
                        mybir.ActivationFunctionType.Identity,
                        bias=bqk_sb[:, bcol:bcol + 1], scale=1.0)
                    qsw = p1.tile([128, 512], BF16, tag="qsw", bufs=2,
                                  name=f"qsw_{n}_{h}_{which}")
                    bsw = bcol + 2 * HG
                    nc.scalar.activation(
                        qsw[0:64, :], ps_acc[64:128, :],
                        mybir.ActivationFunctionType.Identity,
                        bias=bqk_sb[0:64, bsw:bsw + 1], scale=1.0)
                    nc.scalar.activation(
                        qsw[64:128, :], ps_acc[0:64, :],
                        mybir.ActivationFunctionType.Identity,
                        bias=bqk_sb[64:128, bsw:bsw + 1], scale=1.0)
                    # rope: dst = q0*cos + qsw*sin_signed
                    t1 = p1.tile([128, 512], BF16, tag="t1", bufs=2,
                                 name=f"t1_{n}_{h}_{which}")
                    nc.vector.tensor_tensor(t1[:], qsw[:], sin_sb[:],
                                            op=mybir.AluOpType.mult)
                    nc.vector.tensor_tensor(dst_slice, q0[:], cos_sb[:],
                                            op=mybir.AluOpType.mult)
                    nc.vector.tensor_tensor(dst_slice, dst_slice, t1[:],
                                            op=mybir.AluOpType.add)

                for n in range(NS):
                    if n == 1:
                        # out-projection weights ride the sync DMA queue here,
                        # well before phase 2 needs them
                        for dc in range(HG):
                            nc.sync.dma_start(wot[dc][:],
                                              wo[128 * dc:128 * (dc + 1), :])
                    # xT slabs for this chunk; rotating tags prefetch one
                    # slab of the next chunk while this one computes.
                    # xT slabs ride the gpsimd (SWDGE) ring so their slot
                    # waits never head-of-line-block the weight stream on sync
                    xslabs = []
                    for w in range(4):
                        xs = p1.tile([128, 2048], BF16, tag=f"xs{(4 * n + w) % 6}",
                                     bufs=1, name=f"xs_{n}_{w}")
                        base = 8192 * n + 2048 * w
                        if n == 0:
                            # quarter-granularity so the first matmuls start as
                            # soon as their block lands, not the whole slab
                            for q4 in range(4):
                                nc.gpsimd.dma_start(
                                    xs[:, 512 * q4:512 * (q4 + 1)],
                                    xTs[:, base + 512 * q4:base + 512 * (q4 + 1)])
                        else:
                            nc.gpsimd.dma_start(xs[:], xTs[:, base:base + 2048])
                        xslabs.append(xs)

                    def xt_block(ht):
                        s = xslabs[ht // 4]
                        return s[:, 512 * (ht % 4):512 * (ht % 4 + 1)]

                    def qk_pass(which, wdram, evict_accs, evict_base):
                        # one projection pass; optionally interleaves the
                        # previous pass's rope evictions at w boundaries so the
                        # PSUM ring and DVE work stay spread out
                        accs = [ps1.tile([128, 512], F32, tag="pp",
                                         name=f"acc{n}{which}{h}") for h in range(HG)]
                        for w in range(4):
                            wt = load_w_slab(wdram, w, n, f"qk{which}",
                                             split=(n == 0 and which == 0 and w == 0))
                            for hti in range(4):
                                ht = 4 * w + hti
                                for h in range(HG):
                                    nc.tensor.matmul(
                                        accs[h][:],
                                        wt[:, 512 * hti + 128 * h:
                                           512 * hti + 128 * (h + 1)],
                                        xt_block(ht),
                                        start=(ht == 0), stop=(ht == NHT - 1))
                            if evict_accs is not None:
                                dst = kt if evict_base else qt
                                bc = (HG + w) if evict_base else w
                                rope_evict(evict_accs[w],
                                           dst[w][:, 512 * n:512 * (n + 1)],
                                           bc, n, w, evict_base, cos_sb, sin_sb)
                        return accs

                    def load_tables(first_pass_done=None):
                        # constants and rope tables are first needed by the
                        # evictions; keep them off the critical startup path
                        # behind the first weight slabs
                        if n == 0:
                            nc.sync.dma_start(bqk_sb[:], bqk[:])
                            nc.sync.dma_start(od_sb[:], od[:])
                        cos_sb = p1.tile([128, 512], BF16, tag="cos", bufs=2,
                                         name=f"cos_{n}")
                        sin_sb = p1.tile([128, 512], BF16, tag="sin", bufs=2,
                                         name=f"sin_{n}")
                        ns = slice(512 * n, 512 * (n + 1))
                        nc.sync.dma_start(cos_sb[:], cosT[:, ns])
                        nc.sync.dma_start(sin_sb[:], sinTs[:, ns])
                        return cos_sb, sin_sb

                    def v_pass(kaccs_to_evict):
                        # V pass (natural [s, d] layout); K rope evictions (if
                        # any) interleave between weight-slab groups
                        vaccs = [ps1.tile([128, DC], F32, tag="pp",
                                          name=f"vacc{n}{t}") for t in range(4)]
                        for w in range(4):
                            wt = load_w_slab(wvp, w, n, "v")
                            for hti in range(4):
                                ht = 4 * w + hti
                                for t in range(4):
                                    nc.tensor.matmul(
                                        vaccs[t][:],
                                        xt_block(ht)[:, 128 * t:128 * (t + 1)],
                                        wt[:, 512 * hti:512 * (hti + 1)],
                                        start=(ht == 0), stop=(ht == NHT - 1))
                            if kaccs_to_evict is not None:
                                rope_evict(kaccs_to_evict[w],
                                           kt[w][:, 512 * n:512 * (n + 1)],
                                           HG + w, n, w, 1, cos_sb, sin_sb)
                        # V eviction on the Scalar engine (no bias needed: bv
                        # is folded into bo on the host since softmax rows sum
                        # to 1)
                        for t in range(4):
                            nc.scalar.activation(vt[4 * n + t][:], vaccs[t][:],
                                                 mybir.ActivationFunctionType.Copy)

                    if n < NS - 1:
                        # Q pass, K pass, Q evictions, V pass w/ K evictions
                        qaccs = qk_pass(0, wqp, None, 0)
                        cos_sb, sin_sb = load_tables()
                        kaccs = qk_pass(1, wkp, None, 1)
                        for h in range(HG):
                            rope_evict(qaccs[h], qt[h][:, 512 * n:512 * (n + 1)],
                                       h, n, h, 0, cos_sb, sin_sb)
                        v_pass(kaccs)
                    else:
                        # last chunk: K first so kt completes before the V
                        # pass - phase 2's first scores then follow the V
                        # matmuls with no boundary stall
                        kaccs = qk_pass(1, wkp, None, 1)
                        cos_sb, sin_sb = load_tables()
                        qaccs = qk_pass(0, wqp, kaccs, 1)
                        for h in range(HG):
                            rope_evict(qaccs[h], qt[h][:, 512 * n:512 * (n + 1)],
                                       h, n, h, 0, cos_sb, sin_sb)
                        v_pass(None)

            # ---------- phase 2+3: attention + output projection ----------
            with tc.tile_pool(name="ph2", bufs=1) as p2, \
                 tc.tile_pool(name="ph2ps", bufs=1, space="PSUM") as ps2:

                def norm_chain(po, pr, at_h, qc, h):
                    # at = po / rowsum. The DoubleRow rowsum already emitted 64
                    # replicated rows, so the reciprocal reads PSUM directly
                    # and two partition-offset half-multiplies normalize po -
                    # no cross-partition broadcast anywhere
                    recip = p2.tile([64, 512], F32, tag="recip", bufs=2,
                                    name=f"recip{qc}{h}")
                    nc.vector.reciprocal_approx_fast(recip[:], pr[0:64, :])
                    nc.vector.tensor_tensor(at_h[0:64, :], po[0:64, :],
                                            recip[:], op=mybir.AluOpType.mult)
                    nc.vector.tensor_tensor(at_h[64:128, :], po[64:128, :],
                                            recip[:], op=mybir.AluOpType.mult)

                def emit_outproj_group(qc, ti, hc, ats, tag="pf"):
                    # evictions alternate DVE/ACT to keep either queue shallow
                    # one [128 q, 512 cols] block of the out-projection:
                    # 4 matmuls (contraction over this core's 512 d), DVE
                    # eviction, DMA out
                    rs_out = slice(512 * qc + 128 * ti,
                                   512 * qc + 128 * (ti + 1))
                    cs = slice(512 * hc, 512 * (hc + 1))
                    ps_f = ps2.tile([128, 512], F32, tag=tag,
                                    bufs=(1 if tag == "pf" else 2),
                                    name=f"pf{qc}{ti}{hc}")
                    for dc in range(HG):
                        nc.tensor.matmul(
                            ps_f[:], ats[dc][:, 128 * ti:128 * (ti + 1)],
                            wot[dc][:, cs],
                            start=(dc == 0), stop=(dc == HG - 1))
                    ost = p2.tile([128, 512], F32, tag="ost", bufs=3,
                                  name=f"ost{qc}{ti}{hc}")
                    if hc % 2 == 0:
                        nc.vector.tensor_copy(ost[:], ps_f[:])
                    else:
                        nc.scalar.activation(ost[:], ps_f[:],
                                             mybir.ActivationFunctionType.Copy)
                    nc.sync.dma_start(out[rs_out, cs], ost[:])

                # out-projection groups of chunk qc-1 are interleaved into the
                # exp-bound AV stream of chunk qc: the PE gets filler work that
                # never waits on the in-flight exp chain
                def make_emit(qc, h, exps):
                    # scores are emitted two k-tiles ahead of the exp/AV chain
                    # so the PE FIFO always holds work that does not depend on
                    # the in-flight exp
                    qs = slice(512 * qc, 512 * (qc + 1))

                    def emit_scores(kti):
                        ks = slice(128 * kti, 128 * (kti + 1))
                        ps_s = ps2.tile([128, 512], F32, tag="ps", bufs=4,
                                        name=f"pss{qc}{h}{kti}")
                        nc.tensor.matmul(ps_s[:], kt[h][:, ks],
                                         qt[h][:, qs], start=True, stop=True)
                        e = p2.tile([128, 512], BF16, tag="e", bufs=6,
                                    name=f"e{qc}{h}{kti}")
                        nc.scalar.activation(e[:], ps_s[:],
                                             mybir.ActivationFunctionType.Exp,
                                             scale=SCALE,
                                             bias=bqk_sb[:, 4 * HG:4 * HG + 1])
                        exps.append(e)

                    return emit_scores

                opq = []  # deferred out-proj groups: (qc, ti, hc, ats)
                blocks = [(qc, h) for qc in range(NS) for h in range(HG)]
                pending = None  # next block's prefetched (exps, emit)
                ats = []
                for bi, (qc, h) in enumerate(blocks):
                    if h == 0:
                        ats = []
                    hs = slice(128 * h, 128 * (h + 1))
                    ps_o = ps2.tile([128, 512], F32, tag="po", bufs=2,
                                    name=f"po{qc}{h}")
                    ps_r = ps2.tile([64, 512], F32, tag="pr", bufs=1,
                                    name=f"pr{qc}{h}")
                    pks = []
                    if pending is None:
                        exps = []
                        emit_scores = make_emit(qc, h, exps)
                        emit_scores(0)
                        emit_scores(1)
                        emit_scores(2)
                    else:
                        exps, emit_scores = pending
                        pending = None
                    nxt = blocks[bi + 1] if bi + 1 < len(blocks) else None
                    # out-proj filler leads each kti-quad (the exp queue laps
                    # the PE at block starts); h==0 blocks trail it instead -
                    # their groups need the at tile normalized moments earlier
                    op_slot = 3 if h == 0 else 0
                    if True:
                        for kti in range(NKT):
                            if kti + 3 < NKT:
                                emit_scores(kti + 3)
                            elif nxt is not None:
                                # tail slots: prefetch the next block's first
                                # scores so its AV stream starts with hot exps
                                if kti == NKT - 3:
                                    nexps = []
                                    nemit = make_emit(nxt[0], nxt[1], nexps)
                                    nemit(0)
                                    pending = (nexps, nemit)
                                elif kti == NKT - 2:
                                    nemit(1)
                                else:
                                    nemit(2)
                            if kti % 4 == op_slot and opq:
                                emit_outproj_group(*opq.pop(0))
                            e = exps[kti]
                            nc.tensor.matmul(ps_o[:], vt[kti][:, hs], e[:],
                                             start=(kti == 0), stop=(kti == NKT - 1))
                            if kti % 2 == 1:
                                # DVE pair-adds write fp8 into packed slots;
                                # every second pair closes a DoubleRow rowsum
                                # matmul (2 pairs = 256 k per pass, 0.5 cyc/row)
                                j = kti // 2
                                pk_i, slot = divmod(j, 2)
                                if slot == 0:
                                    pks.append(p2.tile(
                                        [128, 2, 512], F8, tag="pk8", bufs=3,
                                        name=f"pk{qc}{h}{pk_i}"))
                                nc.vector.tensor_tensor(
                                    pks[pk_i][:, slot, :], exps[kti - 1][:], e[:],
                                    op=mybir.AluOpType.add)
                                if slot == 1:
                                    nc.tensor.matmul(
                                        ps_r[:], od_sb[:], pks[pk_i][:],
                                        start=(pk_i == 0), stop=(pk_i == 3),
                                        perf_mode=mybir.MatmulPerfMode.DoubleRow)
                    at_h = p2.tile([128, 512], BF16, tag=f"at{h}", bufs=2,
                                   name=f"at{qc}{h}")
                    ats.append(at_h)
                    norm_chain(ps_o, ps_r, at_h, qc, h)
                    if h == HG - 1:
                        opq = [(qc, ti, hc, ats)
                               for ti in range(4) for hc in range(NS)]
                for i, g in enumerate(opq):
                    emit_outproj_group(*g, tag=("pf" if i % 2 == 0 else "po"))

    nc.finalize()
    return nc


_NC = None
LAST_RESULTS = None


def _rope_tables():
    j = np.arange(HD // 2, dtype=np.float64)
    inv_freq = BASE ** (-2.0 * j / HD)
    pos = np.arange(S, dtype=np.float64)
    ang = pos[None, :] * inv_freq[:, None]          # [64, S]
    cos = np.cos(ang)
    sin = np.sin(ang)
    cosT = np.concatenate([cos, cos], axis=0)       # [128, S]
    sinTs = np.concatenate([-sin, sin], axis=0)     # [128, S]
    return (np.ascontiguousarray(cosT.astype(NPBF)),
            np.ascontiguousarray(sinTs.astype(NPBF)))


def _pack_w(w):
    # [H, DC] -> [128, NHT*DC]: block j holds rows 128j..128j+127; each
    # SBUF partition's slab row is contiguous in DRAM
    return np.ascontiguousarray(
        w.reshape(NHT, 128, DC).transpose(1, 0, 2).reshape(128, NHT * DC)
        .astype(NPBF))


def kernel(hidden_state, Wq, bq, Wk, bk, Wv, bv, Wo, bo):
    global _NC, LAST_RESULTS
    hidden_state = np.asarray(hidden_state, dtype=np.float32)
    Wq, bq = np.asarray(Wq, np.float32), np.asarray(bq, np.float32)
    Wk, bk = np.asarray(Wk, np.float32), np.asarray(bk, np.float32)
    Wv, bv = np.asarray(Wv, np.float32), np.asarray(bv, np.float32)
    Wo, bo = np.asarray(Wo, np.float32), np.asarray(bo, np.float32)

    if _NC is None:
        _NC = _build_program()

    cosT, sinTs = _rope_tables()
    perm = np.concatenate([np.arange(0, HD, 2), np.arange(1, HD, 2)])
    od_v = np.ones((128, 2, 64), NPF8)

    in_maps = []
    for c in range(NCORES):
        b, g = divmod(c, HG)
        cols = np.arange(DC) + DC * g
        # per-head de-interleave permutation for Q/K columns
        pcols = np.concatenate([DC * g + HD * h + perm for h in range(HG)])
        bqk_m = np.concatenate([bq[pcols].reshape(HG, HD).T,
                                bk[pcols].reshape(HG, HD).T], axis=1)  # [128, 2*HG]
        # half-swapped copy for the rope-swapped eviction, then the exp
        # bias column (-ln 16, see EXPB)
        bqk_m = np.concatenate(
            [bqk_m, np.concatenate([bqk_m[64:128], bqk_m[0:64]], axis=0),
             np.full((128, 1), EXPB, np.float32)],
            axis=1)  # [128, 4*HG+1]
        # hidden^T packed: [s-chunk n][block j][s within chunk] contiguous
        # per partition: xTs[p, n, j, c] = hidden[b][512n+c, 128j+p]
        xT = hidden_state[b].T                       # [H, S]
        xTs = (xT.reshape(NHT, 128, NS, 512)
               .transpose(1, 2, 0, 3).reshape(128, NS * NHT * 512))
        in_maps.append({
            "xTs": np.ascontiguousarray(xTs.astype(NPBF)),
            "wqp": _pack_w(Wq[:, pcols]),
            "wkp": _pack_w(Wk[:, pcols]),
            "wvp": _pack_w(Wv[:, cols]),
            "bqk": np.ascontiguousarray(bqk_m),
            "cosT": cosT,
            "sinTs": sinTs,
            "od": od_v,
            "wo": np.ascontiguousarray(Wo[cols, :].astype(NPBF)),
        })

    trace = bool(os.environ.get("KERNEL_TRACE"))
    res = bass_utils.run_bass_kernel_spmd(_NC, in_maps, core_ids=list(range(NCORES)),
                                          trace=trace)
    LAST_RESULTS = res

    out = np.zeros((B, S, H), np.float32)
    for c in range(NCORES):
        b = c // HG
        out[b] += res.results[c]["out"]
    # bv's contribution: softmax rows sum to 1, so adding bv to V adds
    # exactly bv @ Wo to every output row
    out += (bo + bv.astype(np.float64) @ Wo.astype(np.float64)).astype(
        np.float32)[None, None, :]
    return out
